# revision 31
# baseline (speedup 1.0000x reference)
"""Trainium2 Bass kernel for nn_DTFDynamicLayer (moe_routing dynamic-token
transformer layer), SPMD across 8 NeuronCores.

kernel(**inputs) takes FULL unsharded numpy inputs (keys as in setup_inputs)
and returns the FULL [B,T,D] output. Sharding (v2, data-parallel):
  - router (scores/topk/positions): token-sharded + tiny AllGathers
  - packed sequence S=2048 split in 8 contiguous blocks of 256 (one per core)
  - Q/K/V projections, RoPE: data-parallel (each core: all 16 heads for its
    own 256 packed tokens), weights in bf16
  - ONE combined AllGather of bf16 K^T+V (4096x256 per core) for attention
  - attention (all heads, own 256 queries over full S), O-proj: local
  - MLP: tensor-parallel over intermediate dim (704/core) over full S in
    bf16, partials combined with f32 ReduceScatter back to own positions
"""
from contextlib import ExitStack

import numpy as np
import ml_dtypes

import concourse.bass as bass
import concourse.mybir as mybir
import concourse.tile as tile
from concourse import bacc
from concourse.bass_utils import run_bass_kernel_spmd
from concourse.masks import make_identity

B, T, D = 2, 2048, 2048
H, HD = 16, 128
I = 5632
EPS = 1e-6
NC = 8
BT = B * T
TOKS = BT // NC          # 512 router tokens per core
K = T // 2               # 1024 selected per batch row
S = B * K                # 2048 packed tokens
SB = S // NC             # 256 packed slots per core
ICOL = I // NC           # 704
DC = D // 128            # 16
SCALE = 1.0 / float(np.sqrt(HD))
IC_CH = [128] * 5 + [ICOL - 5 * 128]   # I-col chunks per core: 5x128 + 64

F32 = mybir.dt.float32
F32R = mybir.dt.float32r
BF16 = mybir.dt.bfloat16
I32 = mybir.dt.int32
AF = mybir.ActivationFunctionType
OP = mybir.AluOpType
P = 128

_NC_CACHE = {}


def _rmsnorm_now(nc, pool, x, out, epst):
    """out = x * rsqrt(mean(x^2)+eps)  ([128, D] token-major, no weight)."""
    sq = pool.tile([P, D], F32, name="rn_sq")
    ssq = pool.tile([P, 1], F32, name="rn_ssq")
    nc.scalar.activation(sq[:], x[:], AF.Square, accum_out=ssq[:])
    rt = pool.tile([P, 1], F32, name="rn_rt")
    nc.scalar.activation(rt[:], ssq[:], AF.Sqrt, scale=1.0 / D,
                         bias=epst[:, :1])
    rec = pool.tile([P, 1], F32, name="rn_rec")
    nc.vector.reciprocal(rec[:], rt[:])
    nc.scalar.activation(out[:], x[:], AF.Copy, scale=rec[:, :1])


def _rope(nc, pool, q, cosT, sinm, out_ap, width):
    """q [128(hd), width] one head, feature-major. out = q*cos + rot(q)*sinm.
    rot(q)[0:64]=q[64:128], rot(q)[64:128]=q[0:64]; sinm rows 0:64 pre-negated.
    out_ap dtype may differ (e.g. bf16) - converted on the final add."""
    rot = pool.tile([P, width], F32, name="rp_rot", tag="rp_rot")
    nc.vector.tensor_copy(rot[0:64, :], q[64:P, :])
    nc.vector.tensor_copy(rot[64:P, :], q[0:64, :])
    t1 = pool.tile([P, width], F32, name="rp_t1", tag="rp_t1")
    nc.vector.tensor_mul(t1[:], q[:], cosT[:, :width])
    t2 = pool.tile([P, width], F32, name="rp_t2", tag="rp_t2")
    nc.vector.tensor_mul(t2[:], rot[:], sinm[:, :width])
    nc.vector.tensor_add(out_ap, t1[:], t2[:])


def _gather_cossin_T(nc, pool, ppool, ident, cosf, sinf, rows_col, cosT, sinm,
                     col_off):
    """Gather cos/sin rows (128 of them, by rows_col int32 [128,1]) and write
    transposed into cosT/sinm at column offset col_off. sinm rows 0:64 negated.
    """
    for (src, dstT, negate) in ((cosf, cosT, False), (sinf, sinm, True)):
        g = pool.tile([P, HD], F32, name="cs_g", tag="cs_g")
        nc.gpsimd.indirect_dma_start(
            out=g[:], out_offset=None, in_=src[:],
            in_offset=bass.IndirectOffsetOnAxis(ap=rows_col, axis=0))
        pt = ppool.tile([P, P], F32, space="PSUM", name="cs_p", tag="cs_p")
        nc.tensor.transpose(pt[:], g[:], ident[:])
        sl = slice(col_off, col_off + P)
        if negate:
            nc.scalar.activation(dstT[0:64, sl], pt[0:64, :], AF.Copy,
                                 scale=-1.0)
            nc.scalar.activation(dstT[64:P, sl], pt[64:P, :], AF.Copy)
        else:
            nc.vector.tensor_copy(dstT[:, sl], pt[:])


def build(phases="full"):
    nc = bacc.Bacc(None, target_bir_lowering=False)
    _build(nc, phases)
    nc.finalize()
    return nc


def _build(nc, phases):
    dp = nc.declare_dram_parameter
    orig_s = dp("orig_s", [TOKS, D], F32, isOutput=False)
    post_s = dp("post_s", [TOKS, D], F32, isOutput=False)
    prior_s = dp("prior_s", [TOKS, D], F32, isOutput=False)
    hidden = dp("hidden", [BT, D], F32, isOutput=False)
    cosf = dp("cosf", [BT, HD], F32, isOutput=False)
    sinf = dp("sinf", [BT, HD], F32, isOutput=False)
    qw = dp("qw", [D, H * HD], BF16, isOutput=False)
    kw = dp("kw", [D, H * HD], BF16, isOutput=False)
    vw = dp("vw", [D, H * HD], BF16, isOutput=False)
    qb = dp("qb", [H * HD, 1], F32, isOutput=False)
    kb = dp("kb", [H * HD, 1], F32, isOutput=False)
    vb = dp("vb", [H * HD, 1], F32, isOutput=False)
    ow = dp("ow", [H * HD, D], F32, isOutput=False)
    ln1w = dp("ln1w", [D, 1], F32, isOutput=False)
    ln2w = dp("ln2w", [D, 1], F32, isOutput=False)
    gatew = dp("gatew", [D, I], BF16, isOutput=False)
    upw = dp("upw", [D, I], BF16, isOutput=False)
    downw = dp("downw", [I, D], BF16, isOutput=False)
    # cconst: [beta_cu, beta_ce, beta_ce*ce_off, i0(=c*SB), unused,
    #          unused, i0row(=(c%4)*TOKS), b(=c//4)]
    cconst = dp("cconst", [1, 8], F32, isOutput=False)

    upd_out = dp("upd_out", [SB, D], F32, isOutput=True)
    selidx_out = dp("selidx_out", [SB, 1], I32, isOutput=True)
    dbg = dp("dbg", [P, 16], F32, isOutput=True)

    RG = [list(range(NC))]

    with tile.TileContext(nc) as tc, ExitStack() as es:
        # -------- DRAM internals (pool tiles => dep tracking) --------
        dr = es.enter_context(tc.tile_pool(name="dram", bufs=1, space="DRAM"))

        def dtile(name, shape, dtype=F32, shared=False):
            return dr.tile(shape, dtype, name=name,
                           addr_space="Shared" if shared else "Local")

        sc_in = dtile("sc_in", [TOKS, 1])
        sc_all = dtile("sc_all", [BT, 1], shared=True)
        mk_in = dtile("mk_in", [TOKS, 1])
        mk_all = dtile("mk_all", [BT, 1], shared=True)
        ps_in = dtile("ps_in", [TOKS, 1])
        ps_all = dtile("ps_all", [BT, 1], shared=True)
        selidx_d = dtile("selidx_d", [S + P, 1], I32)
        # kv_in rows 0..2047: K^T own (row h*128+d, col own token)
        # rows 2048..4095: V own [256 tok, 2048 hd] viewed as [2048, 256]
        kv_in = dtile("kv_in", [2 * H * HD, SB], BF16)
        kv_all = dtile("kv_all", [NC * 2 * H * HD, SB], BF16, shared=True)

        # -------- persistent SBUF --------
        pers = es.enter_context(tc.tile_pool(name="pers", bufs=1))
        ident = pers.tile([P, P], F32)
        make_identity(nc, ident[:])
        cc_sb = pers.tile([1, 8], F32)
        nc.sync.dma_start(out=cc_sb[:], in_=cconst[:])
        ccb = pers.tile([P, 8], F32)
        nc.gpsimd.partition_broadcast(ccb[:], cc_sb[:])
        col_bcu = ccb[:, 0:1]
        col_bce = ccb[:, 1:2]
        col_ceo = ccb[:, 2:3]
        col_i0 = ccb[:, 3:4]
        col_i0row = ccb[:, 6:7]
        col_b = ccb[:, 7:8]
        epst = pers.tile([P, 1], F32)
        nc.vector.memset(epst[:], EPS)
        iota_pf = pers.tile([P, SB], F32)      # value = p - f
        _it = pers.tile([P, SB], I32)
        nc.gpsimd.iota(_it[:], pattern=[[-1, SB]], base=0, channel_multiplier=1)
        nc.vector.tensor_copy(iota_pf[:], _it[:])
        iota_jmp = pers.tile([P, T], F32)      # value = j - p
        _it2 = pers.tile([P, T], I32)
        nc.gpsimd.iota(_it2[:], pattern=[[1, T]], base=0, channel_multiplier=-1)
        nc.vector.tensor_copy(iota_jmp[:], _it2[:])
        lnw_cols = pers.tile([P, 2 * DC], F32)  # [:, 0:16]=ln1, [:,16:32]=ln2
        nc.sync.dma_start(out=lnw_cols[:, 0:DC],
                          in_=ln1w.rearrange("(d p) one -> p d one", p=P))
        nc.sync.dma_start(out=lnw_cols[:, DC:2 * DC],
                          in_=ln2w.rearrange("(d p) one -> p d one", p=P))
        dbg_t = pers.tile([P, 16], F32)
        nc.vector.memset(dbg_t[:], 0.0)

        s_cols = [pers.tile([P, 1], F32, name=f"s_col{t}") for t in range(4)]
        m_cols = [pers.tile([P, 1], F32, name=f"m_col{t}") for t in range(4)]
        p_cols = [pers.tile([P, 1], F32, name=f"p_col{t}") for t in range(4)]

        # ============ Phase R1: scores for own 512 tokens ============
        with tc.tile_pool(name="router", bufs=2) as rp:
            for t in range(4):
                cu = rp.tile([P, 1], F32, name="cu")
                ce = rp.tile([P, 1], F32, name="ce")
                for (a_ap, b_ap, dst) in ((orig_s, post_s, cu),
                                          (post_s, prior_s, ce)):
                    at = rp.tile([P, D], F32, name="r_at")
                    bt = rp.tile([P, D], F32, name="r_bt")
                    nc.sync.dma_start(out=at[:], in_=a_ap[t * P:(t + 1) * P, :])
                    nc.sync.dma_start(out=bt[:], in_=b_ap[t * P:(t + 1) * P, :])
                    df = rp.tile([P, D], F32, name="r_df")
                    nc.vector.tensor_sub(df[:], at[:], bt[:])
                    sq = rp.tile([P, D], F32, name="r_sq")
                    ssq = rp.tile([P, 1], F32, name="r_ssq")
                    nc.scalar.activation(sq[:], df[:], AF.Square,
                                         accum_out=ssq[:])
                    nc.scalar.activation(dst[:], ssq[:], AF.Sqrt)
                t1 = rp.tile([P, 1], F32, name="r_t1")
                nc.vector.tensor_scalar(t1[:], cu[:], col_bcu, None,
                                        op0=OP.mult)
                nc.vector.scalar_tensor_tensor(
                    s_cols[t][:], in0=ce[:], scalar=col_bce, in1=t1[:],
                    op0=OP.mult, op1=OP.add)
                nc.vector.tensor_scalar(s_cols[t][:], s_cols[t][:], col_ceo,
                                        None, op0=OP.add)
            sc_flat = rp.tile([P, 4], F32, name="scflat")
            for t in range(4):
                nc.vector.tensor_copy(sc_flat[:, t:t + 1], s_cols[t][:])
            nc.sync.dma_start(
                out=sc_in.rearrange("(t p) one -> p t one", p=P),
                in_=sc_flat[:])
        nc.gpsimd.collective_compute("AllGather", OP.bypass, replica_groups=RG,
                                     ins=[sc_in[:]], outs=[sc_all[:]])

        if phases == "score":
            with tc.tile_pool(name="sfin", bufs=1) as fp:
                sall = fp.tile([P, BT // P], F32, name="sall")
                nc.sync.dma_start(
                    out=sall[:],
                    in_=sc_all.rearrange("(t p) one -> p t one", p=P))
                nc.vector.tensor_copy(dbg_t[:, 0:1], sall[:, 0:1])
                nc.vector.tensor_copy(dbg_t[:, 1:2], sall[:, 31:32])
                nc.vector.tensor_copy(dbg_t[:, 2:3], s_cols[0][:])
                nc.sync.dma_start(out=dbg[:], in_=dbg_t[:])
            return

        # ============ Phase R2: rank -> mask for own tokens ============
        # rank_i = #{j: s_j>s_i} + #{j<i: s_j==s_i} = (T - sum(le)) + sum(eq*jlt)
        # mask = rank <= K-1  <=>  acc = sum(le) - sum(eq*jlt) >= T-K+1
        with tc.tile_pool(name="rank", bufs=2) as rp:
            sbr = rp.tile([P, T], F32, name="sbr")
            _row_select_bcast(nc, rp, sc_all, col_b, sbr)
            for t in range(4):
                # no-tie rank: rank_i = T - sum(le); random f32 scores make
                # exact duplicates measure-zero, so tie-break terms dropped
                le = rp.tile([P, T], F32, name="k_le")
                nc.vector.tensor_scalar(le[:], sbr[:], s_cols[t][:, :1], None,
                                        op0=OP.is_le)
                acc = rp.tile([P, 1], F32, name="k_acc")
                nc.vector.tensor_reduce(acc[:], le[:],
                                        axis=mybir.AxisListType.X, op=OP.add)
                # mask = acc >= T-K+1  <=>  (-acc) <= -(T-K+1)
                nacc = rp.tile([P, 1], F32, name="k_nacc")
                nc.vector.tensor_scalar_mul(nacc[:], acc[:], -1.0)
                nc.vector.tensor_scalar(m_cols[t][:], nacc[:],
                                        float(-(T - K + 1)), None,
                                        op0=OP.is_le)
                if t == 0:
                    nc.vector.tensor_copy(dbg_t[:, 0:1], acc[:])
                    nc.vector.tensor_copy(dbg_t[:, 1:2], m_cols[t][:])
                    nc.vector.tensor_copy(dbg_t[:, 2:3], s_cols[t][:])
            mflat = rp.tile([P, 4], F32, name="mflat")
            for t in range(4):
                nc.vector.tensor_copy(mflat[:, t:t + 1], m_cols[t][:])
            nc.sync.dma_start(
                out=mk_in.rearrange("(t p) one -> p t one", p=P), in_=mflat[:])
        nc.gpsimd.collective_compute("AllGather", OP.bypass, replica_groups=RG,
                                     ins=[mk_in[:]], outs=[mk_all[:]])

        if phases == "rank":
            with tc.tile_pool(name="kfin", bufs=1) as fp:
                mall = fp.tile([P, BT // P], F32, name="mall")
                nc.sync.dma_start(
                    out=mall[:],
                    in_=mk_all.rearrange("(t p) one -> p t one", p=P))
                nc.vector.tensor_copy(dbg_t[:, 4:5], mall[:, 0:1])
                nc.vector.tensor_copy(dbg_t[:, 5:6], mall[:, 31:32])
                nc.sync.dma_start(out=dbg[:], in_=dbg_t[:])
            return

        # ============ Phase R3: positions ============
        with tc.tile_pool(name="pos", bufs=2) as rp:
            mbr = rp.tile([P, T], F32, name="mbr")
            _row_select_bcast(nc, rp, mk_all, col_b, mbr)
            for t in range(4):
                jlt = rp.tile([P, T], F32, name="p_jlt")
                rhs = rp.tile([P, 1], F32, name="p_rhs")
                nc.vector.tensor_scalar(rhs[:], col_i0row, float(t * P - 1),
                                        None, op0=OP.add)
                nc.vector.tensor_scalar(jlt[:], iota_jmp[:], rhs[:, :1], None,
                                        op0=OP.is_le)
                mj = rp.tile([P, T], F32, name="p_mj")
                nc.vector.tensor_mul(mj[:], mbr[:], jlt[:])
                nc.vector.tensor_reduce(p_cols[t][:], mj[:],
                                        axis=mybir.AxisListType.X, op=OP.add)
                if t == 0:
                    nc.vector.tensor_copy(dbg_t[:, 3:4], p_cols[t][:])
            pflat = rp.tile([P, 4], F32, name="pflat")
            for t in range(4):
                nc.vector.tensor_copy(pflat[:, t:t + 1], p_cols[t][:])
            nc.sync.dma_start(
                out=ps_in.rearrange("(t p) one -> p t one", p=P), in_=pflat[:])
        nc.gpsimd.collective_compute("AllGather", OP.bypass, replica_groups=RG,
                                     ins=[ps_in[:]], outs=[ps_all[:]])

        if phases == "pos":
            with tc.tile_pool(name="pfin", bufs=1) as fp:
                pall = fp.tile([P, BT // P], F32, name="pall")
                nc.sync.dma_start(
                    out=pall[:],
                    in_=ps_all.rearrange("(t p) one -> p t one", p=P))
                nc.vector.tensor_copy(dbg_t[:, 4:5], pall[:, 0:1])
                nc.vector.tensor_copy(dbg_t[:, 5:6], pall[:, 31:32])
                nc.sync.dma_start(out=dbg[:], in_=dbg_t[:])
            return

        # ============ Phase SCT: slot -> flat row map ============
        with tc.tile_pool(name="scat", bufs=4) as sp:
            zt = sp.tile([P, (S + P) // P], I32, name="sc_zero")
            nc.vector.memset(zt[:], 0)
            nc.sync.dma_start(
                out=selidx_d.rearrange("(t p) one -> p t one", p=P), in_=zt[:])
            mk_t = sp.tile([P, BT // P], F32, name="mk_t")
            ps_t = sp.tile([P, BT // P], F32, name="ps_t")
            nc.sync.dma_start(out=mk_t[:],
                              in_=mk_all.rearrange("(t p) one -> p t one", p=P))
            nc.sync.dma_start(out=ps_t[:],
                              in_=ps_all.rearrange("(t p) one -> p t one", p=P))
            dump_i = sp.tile([P, 1], I32, name="sc_dumpi")
            nc.gpsimd.iota(dump_i[:], pattern=[[0, 1]], base=S,
                           channel_multiplier=1)
            dump_f = sp.tile([P, 1], F32, name="sc_dumpf")
            nc.vector.tensor_copy(dump_f[:], dump_i[:])
            dump_ni = sp.tile([P, 1], I32, name="sc_dumpni")
            nc.gpsimd.iota(dump_ni[:], pattern=[[0, 1]], base=-S,
                           channel_multiplier=-1)
            dump_nf = sp.tile([P, 1], F32, name="sc_dumpnf")
            nc.vector.tensor_copy(dump_nf[:], dump_ni[:])
            # batched slot computation over all 32 chunks:
            # slot' = m*(pos + b*K - (S+p)) + (S+p)  (per-part dump row)
            NCH = BT // P
            t1 = sp.tile([P, NCH], F32, name="sc_t1")
            for b in range(2):
                hsl = slice(b * (NCH // 2), (b + 1) * (NCH // 2))
                nc.vector.tensor_scalar(t1[:, hsl], ps_t[:, hsl],
                                        float(b * K), None, op0=OP.add)
            nc.vector.tensor_scalar(t1[:], t1[:], dump_nf[:, :1], None,
                                    op0=OP.add)
            t2 = sp.tile([P, NCH], F32, name="sc_t2")
            nc.vector.tensor_mul(t2[:], t1[:], mk_t[:])
            nc.vector.tensor_scalar(t2[:], t2[:], dump_f[:, :1], None,
                                    op0=OP.add)
            off_i = sp.tile([P, NCH], I32, name="sc_off")
            nc.vector.tensor_copy(off_i[:], t2[:])
            val_i = sp.tile([P, NCH], I32, name="sc_val")
            nc.gpsimd.iota(val_i[:], pattern=[[P, NCH]], base=0,
                           channel_multiplier=1)
            for t in range(NCH):
                nc.gpsimd.indirect_dma_start(
                    out=selidx_d[:],
                    out_offset=bass.IndirectOffsetOnAxis(ap=off_i[:, t:t + 1],
                                                         axis=0),
                    in_=val_i[:, t:t + 1], in_offset=None)

        # ============ Phase G: gathers ============
        gpL = es.enter_context(tc.tile_pool(name="gpL", bufs=1))   # long-lived
        own_rows = []
        selh = []
        gate_g = []
        myslot = gpL.tile([P, 2], I32)
        _si = gpL.tile([P, 2], I32)
        _slotf = gpL.tile([P, 2], F32)
        for half in range(2):
            nc.gpsimd.iota(_si[:, half:half + 1], pattern=[[0, 1]],
                           base=half * P, channel_multiplier=1)
        nc.vector.tensor_copy(_slotf[:], _si[:])
        for half in range(2):
            nc.vector.tensor_scalar(_slotf[:, half:half + 1],
                                    _slotf[:, half:half + 1], col_i0, None,
                                    op0=OP.add)
        nc.vector.tensor_copy(myslot[:], _slotf[:])
        for half in range(2):
            orow = gpL.tile([P, 1], I32, name=f"orow{half}")
            nc.gpsimd.indirect_dma_start(
                out=orow[:], out_offset=None, in_=selidx_d[:],
                in_offset=bass.IndirectOffsetOnAxis(
                    ap=myslot[:, half:half + 1], axis=0))
            own_rows.append(orow)
            sh = gpL.tile([P, D], F32, name=f"selh{half}")
            nc.gpsimd.indirect_dma_start(
                out=sh[:], out_offset=None, in_=hidden[:],
                in_offset=bass.IndirectOffsetOnAxis(ap=orow[:, :1], axis=0),
                bounds_check=BT - 1, oob_is_err=False)
            selh.append(sh)
            ssc = gpL.tile([P, 1], F32, name=f"ssc{half}")
            nc.gpsimd.indirect_dma_start(
                out=ssc[:], out_offset=None, in_=sc_all[:],
                in_offset=bass.IndirectOffsetOnAxis(ap=orow[:, :1], axis=0))
            gg = gpL.tile([P, 1], F32, name=f"gate{half}")
            nc.scalar.activation(gg[:], ssc[:], AF.Sigmoid)
            gate_g.append(gg)
        x1 = [gpL.tile([P, D], F32, name=f"x1_{i}") for i in range(2)]

        if phases == "full":
            # attention-lived pool (opened before gpQ: LIFO close order)
            esA = ExitStack()
            gpA = esA.enter_context(tc.tile_pool(name="gpA", bufs=1))
            q_own = gpA.tile([P, H, SB], BF16)
            o_fm = gpA.tile([P, H, SB], F32R)

            # mid-lived pool: through QKV
            esQ = ExitStack()
            gpQ = esQ.enter_context(tc.tile_pool(name="gpQ", bufs=1))
            cosT_o = gpQ.tile([P, SB], F32)
            sinm_o = gpQ.tile([P, SB], F32)
            h1T_own = gpQ.tile([P, DC, SB], BF16)
            vt_blk = gpQ.tile([P, 2, H * HD], BF16)
            with tc.tile_pool(name="cso", bufs=3) as cp, \
                 tc.tile_pool(name="csop", bufs=4, space="PSUM") as cpp:
                for half in range(2):
                    _gather_cossin_T(nc, cp, cpp, ident, cosf, sinf,
                                     own_rows[half][:, :1], cosT_o, sinm_o,
                                     half * P)

            # ============ Phase N1: h1 = rmsnorm(selh); h1T own (bf16) ======
            with tc.tile_pool(name="n1", bufs=2) as np_, \
                 tc.tile_pool(name="n1p", bufs=4, space="PSUM") as npp:
                for half in range(2):
                    h1 = np_.tile([P, D], F32, name="h1")
                    _rmsnorm_now(nc, np_, selh[half], h1, epst)
                    for d in range(DC):
                        pt = npp.tile([P, P], F32, space="PSUM", name="n1_tp")
                        nc.tensor.transpose(pt[:], h1[:, d * P:(d + 1) * P],
                                            ident[:])
                        # fold ln1 weight (per-d-row) into the PSUM->SBUF copy
                        nc.vector.tensor_scalar(
                            h1T_own[:, d, half * P:(half + 1) * P],
                            pt[:], lnw_cols[:, d:d + 1], None, op0=OP.mult)

            # ============ Phase QKV (DP: all heads, own 256 tokens) ========
            with tc.tile_pool(name="qkv", bufs=2) as qp, \
                 tc.tile_pool(name="qkvw", bufs=1) as wpool:

                def _proj(wap, b_ap, w_i, qpp, qtp):
                    wt = wpool.tile([P, DC, H * HD], BF16, name="wt",
                                    tag="wt")
                    nc.sync.dma_start(
                        out=wt[:], in_=wap.rearrange("(d p) c -> p d c", p=P))
                    bias = qp.tile([P, H], F32, name="bias", tag="bias")
                    nc.sync.dma_start(
                        out=bias[:],
                        in_=b_ap.rearrange("(h p) one -> p h one", p=P))
                    for g4 in range(4):
                        ps4 = [qpp.tile([P, SB], F32, space="PSUM",
                                        name=f"qkv_ps{j}", tag=f"qkv_ps{j}")
                               for j in range(4)]
                        for d in range(DC):
                            for j in range(4):
                                oc = g4 * 4 + j
                                nc.tensor.matmul(
                                    ps4[j][:], wt[:, d, oc * P:(oc + 1) * P],
                                    h1T_own[:, d, :], start=(d == 0),
                                    stop=(d == DC - 1))
                        for j in range(4):
                            oc = g4 * 4 + j
                            hq = qp.tile([P, SB], F32, name="qkv_h",
                                         tag="qkv_h")
                            nc.scalar.activation(hq[:], ps4[j][:], AF.Identity,
                                                 bias=bias[:, oc:oc + 1])
                            if w_i == 0:
                                _rope(nc, qp, hq, cosT_o, sinm_o,
                                      q_own[:, oc, :], SB)
                            elif w_i == 1:
                                kr = qp.tile([P, SB], BF16, name="kr",
                                             tag="kr")
                                _rope(nc, qp, hq, cosT_o, sinm_o, kr[:], SB)
                                nc.sync.dma_start(
                                    out=kv_in[oc * P:(oc + 1) * P, :],
                                    in_=kr[:])
                            else:
                                for half in range(2):
                                    ptt = qtp.tile([P, P], F32, space="PSUM",
                                                   name="v_tp", tag="v_tp")
                                    nc.tensor.transpose(
                                        ptt[:], hq[:, half * P:(half + 1) * P],
                                        ident[:])
                                    nc.vector.tensor_copy(
                                        vt_blk[:, half, oc * P:(oc + 1) * P],
                                        ptt[:])

                with tc.tile_pool(name="qkvp2", bufs=2, space="PSUM") as qpp2:
                    _proj(kw, kb, 1, qpp2, None)
                with tc.tile_pool(name="qkvp1", bufs=1, space="PSUM") as qpp1, \
                     tc.tile_pool(name="qkvtp", bufs=4, space="PSUM") as qtp:
                    _proj(vw, vb, 2, qpp1, qtp)
                for half in range(2):
                    nc.sync.dma_start(
                        out=kv_in[2048 + half * 1024:
                                  2048 + (half + 1) * 1024, :].rearrange(
                                      "(p g) c -> p (g c)", p=P),
                        in_=vt_blk[:, half, :])
                # K/V shipped; Q-proj below overlaps the AllGather
                nc.gpsimd.collective_compute("AllGather", OP.bypass,
                                             replica_groups=RG,
                                             ins=[kv_in[:]], outs=[kv_all[:]])
                with tc.tile_pool(name="qkvp3", bufs=2, space="PSUM") as qpp3:
                    _proj(qw, qb, 0, qpp3, None)
            esQ.close()

            # OPROJ weight prefetch: first 2 groups load during the AG/ATT
            # window (DMA engines are otherwise idle during the collective)
            op2 = esA.enter_context(tc.tile_pool(name="opj2", bufs=2))
            ow_view = ow.rearrange("(x p) c -> p x c", p=P)
            ow_tiles = {}
            for grp in range(2):
                og = op2.tile([P, H, 4 * P], F32R, name="ow_g", tag="ow_g")
                nc.sync.dma_start(
                    out=og[:],
                    in_=ow_view[:, :, grp * 4 * P:(grp + 1) * 4 * P].bitcast(
                        F32R))
                ow_tiles[grp] = og

            # ============ Phase ATT ============
            with tc.tile_pool(name="att", bufs=2) as ap, \
                 tc.tile_pool(name="att1", bufs=1) as ap1, \
                 tc.tile_pool(name="attp", bufs=2, space="PSUM") as app:
                # causal masks per j-chunk: keep (p - f) <= i0 - jc*128
                # (no dep on kv_all - scheduler overlaps this with the AG)
                masks = ap1.tile([P, S // P, SB], BF16, name="masks")
                for jc in range(S // P):
                    rhsc = ap.tile([P, 1], F32, name="a_rhs")
                    nc.vector.tensor_scalar(rhsc[:], col_i0, float(-jc * P),
                                            None, op0=OP.add)
                    nc.vector.tensor_scalar(masks[:, jc, :], iota_pf[:],
                                            rhsc[:, :1], None, op0=OP.is_le)
                onesb = ap1.tile([P, 1], BF16, name="onesb")
                nc.vector.memset(onesb[:], 1.0)
                kv_view = kv_all.rearrange("(c x) s -> x c s", c=NC)
                vv_view = kv_all.rearrange(
                    "(c z tc p g) s -> z g p c tc s",
                    c=NC, z=2, tc=2, p=P, g=8)
                for h in range(H):
                    khead = ap.tile([P, NC, SB], BF16, name="khead")
                    nc.sync.dma_start(
                        out=khead[:], in_=kv_view[h * P:(h + 1) * P, :, :])
                    if h % 2 == 0:
                        # one load covers this head pair (512B elems, no
                        # sub-512B DMA penalty)
                        vpair = ap.tile([P, 2, NC, 2 * P], BF16, name="vpair",
                                        tag="vpair")
                        for tcv in range(2):
                            nc.sync.dma_start(
                                out=vpair[:, tcv, :, :],
                                in_=vv_view[1, h // 2, :, :, tcv, :])
                    hc0 = (h % 2) * P
                    psum_o = app.tile([P, SB], F32, space="PSUM", name="a_po")
                    psum_s = app.tile([1, SB], F32, space="PSUM", name="a_ps")
                    for jp in range(S // P // 2):
                        # paired k-chunks: 2 QK matmuls -> one 512-wide
                        # exp+mask (Act is the steady-state bottleneck)
                        pa2 = app.tile([P, 2 * SB], F32, space="PSUM",
                                       name="a_pa")
                        for u in range(2):
                            jc = 2 * jp + u
                            nc.tensor.matmul(
                                pa2[:, u * SB:(u + 1) * SB],
                                khead[:, jc // 2,
                                      (jc % 2) * P:(jc % 2 + 1) * P],
                                q_own[:, h, :], start=True, stop=True,
                                skip_group_check=True)
                        et = ap.tile([P, 2 * SB], BF16, name="a_et")
                        nc.scalar.activation(et[:], pa2[:], AF.Exp,
                                             scale=SCALE)
                        ex = ap.tile([P, 2 * SB], BF16, name="a_ex")
                        nc.vector.tensor_mul(ex[:], et[:],
                                             masks[:, 2 * jp:2 * jp + 2, :])
                        for u in range(2):
                            jc = 2 * jp + u
                            exs = ex[:, u * SB:(u + 1) * SB]
                            nc.tensor.matmul(psum_s[:], onesb[:], exs,
                                             start=(jc == 0),
                                             stop=(jc == S // P - 1),
                                             skip_group_check=True)
                            nc.tensor.matmul(psum_o[:],
                                             vpair[:, jc % 2, jc // 2,
                                                   hc0:hc0 + P],
                                             exs,
                                             start=(jc == 0),
                                             stop=(jc == S // P - 1),
                                             skip_group_check=True)
                    rec = ap.tile([1, SB], F32, name="a_rec")
                    nc.vector.reciprocal(rec[:], psum_s[:])
                    recb = ap.tile([P, SB], F32, name="a_recb")
                    nc.gpsimd.partition_broadcast(recb[:], rec[:])
                    nc.vector.tensor_mul(o_fm[:, h, :], psum_o[:].bitcast(F32R),
                                         recb[:].bitcast(F32R))

            # ============ Phase OPROJ: x1 = selh + ow.T @ o_fm ============
            with tc.tile_pool(name="opj", bufs=1) as op_, \
                 tc.tile_pool(name="opjp", bufs=3, space="PSUM") as opp:
                for grp in range(4):      # D col groups of 512
                    if grp in ow_tiles:
                        ow_g = ow_tiles[grp]
                    else:
                        ow_g = op2.tile([P, H, 4 * P], F32R, name="ow_g",
                                        tag="ow_g")
                        nc.sync.dma_start(
                            out=ow_g[:],
                            in_=ow_view[:, :,
                                        grp * 4 * P:(grp + 1) * 4 * P].bitcast(
                                            F32R))
                    for dd in range(4):
                        d = grp * 4 + dd
                        pt = opp.tile([P, SB], F32, space="PSUM", name="o_ps")
                        for h in range(H):
                            nc.tensor.matmul(pt[:],
                                             ow_g[:, h, dd * P:(dd + 1) * P],
                                             o_fm[:, h, :], start=(h == 0),
                                             stop=(h == H - 1))
                        for half in range(2):
                            ot = op_.tile([P, P], F32, name="o_sb")
                            nc.vector.tensor_copy(ot[:],
                                                  pt[:, half * P:(half + 1) * P])
                            pt2 = opp.tile([P, P], F32, space="PSUM", name="o_tp")
                            nc.tensor.transpose(pt2[:], ot[:], ident[:])
                            nc.vector.tensor_add(
                                x1[half][:, d * P:(d + 1) * P], pt2[:],
                                selh[half][:, d * P:(d + 1) * P])
            esA.close()

            # ============ Phase N2 + MLP (DP: own 256 tokens, bf16) ========
            NIG = I // 512            # 11 i-groups of 512
            with tc.tile_pool(name="mlp", bufs=1) as mp, \
                 tc.tile_pool(name="mlps", bufs=1) as mps, \
                 tc.tile_pool(name="mlp2", bufs=2) as mp2:
                h2T_own = mp.tile([P, DC, SB], BF16, name="h2T_own")
                with tc.tile_pool(name="n2p", bufs=4, space="PSUM") as n2p:
                    for half in range(2):
                        h2 = mps.tile([P, D], F32, name="h2", tag="h2")
                        _rmsnorm_now(nc, mps, x1[half], h2, epst)
                        for d in range(DC):
                            pt = n2p.tile([P, P], F32, space="PSUM",
                                          name="m_tp")
                            nc.tensor.transpose(pt[:], h2[:, d * P:(d + 1) * P],
                                                ident[:])
                            nc.scalar.activation(
                                h2T_own[:, d, half * P:(half + 1) * P], pt[:],
                                AF.Copy, scale=lnw_cols[:, DC + d:DC + d + 1])
                act = mp.tile([P, I // P, SB], BF16, name="act")
                gw_view = gatew.rearrange("(d p) c -> p d c", p=P)
                uw_view = upw.rearrange("(d p) c -> p d c", p=P)
                with tc.tile_pool(name="mlpg", bufs=2) as mpg, \
                     tc.tile_pool(name="mlpp", bufs=2, space="PSUM") as mpp:
                    for ig in range(NIG):
                        isl = slice(ig * 512, (ig + 1) * 512)
                        gwt = mpg.tile([P, DC, 512], BF16, name="gwt",
                                       tag="gwt")
                        nc.sync.dma_start(out=gwt[:], in_=gw_view[:, :, isl])
                        uwt = mpg.tile([P, DC, 512], BF16, name="uwt",
                                       tag="uwt")
                        nc.sync.dma_start(out=uwt[:], in_=uw_view[:, :, isl])
                        for i4 in range(4):
                            ic = ig * 4 + i4
                            ptg = mpp.tile([P, SB], F32, space="PSUM",
                                           name="m_ptg", tag="m_ptg")
                            ptu = mpp.tile([P, SB], F32, space="PSUM",
                                           name="m_ptu", tag="m_ptu")
                            for d in range(DC):
                                nc.tensor.matmul(
                                    ptg[:], gwt[:, d, i4 * P:(i4 + 1) * P],
                                    h2T_own[:, d, :],
                                    start=(d == 0), stop=(d == DC - 1))
                            for d in range(DC):
                                nc.tensor.matmul(
                                    ptu[:], uwt[:, d, i4 * P:(i4 + 1) * P],
                                    h2T_own[:, d, :],
                                    start=(d == 0), stop=(d == DC - 1))
                            sg = mpg.tile([P, SB], F32, name="m_sg",
                                          tag="m_sg")
                            nc.scalar.activation(sg[:], ptg[:], AF.Silu)
                            nc.vector.tensor_mul(act[:, ic, :], sg[:], ptu[:])
                # down-proj (DP): accumulate own [256, D] across all 44 ic
                dw_view = downw.rearrange("(g p) c -> p g c", p=P)
                with tc.tile_pool(name="mlpd", bufs=1, space="PSUM") as dpp, \
                     tc.tile_pool(name="fin", bufs=2) as fp:
                    dps = [dpp.tile([P, 4 * P], F32, space="PSUM",
                                    name=f"dps{t}")
                           for t in range(8)]
                    for ig in range(NIG):
                        dwn = mp2.tile([P, 4, D], BF16, name="dwn", tag="dwn")
                        nc.sync.dma_start(
                            out=dwn[:],
                            in_=dw_view[:, ig * 4:(ig + 1) * 4, :])
                        for tch in range(2):
                            for dgrp in range(4):
                                ps = dps[tch * 4 + dgrp]
                                for i4 in range(4):
                                    ic = ig * 4 + i4
                                    nc.tensor.matmul(
                                        ps[:],
                                        act[:, ic, tch * P:(tch + 1) * P],
                                        dwn[:, i4,
                                            dgrp * 512:(dgrp + 1) * 512],
                                        start=(ig == 0 and i4 == 0),
                                        stop=(ig == NIG - 1 and i4 == 3),
                                        skip_group_check=True)
                    # ===== final gated update straight out of PSUM =====
                    for tch in range(2):
                        upd = fp.tile([P, D], F32, name="f_upd", tag="f_upd")
                        for dgrp in range(4):
                            sl = slice(dgrp * 512, (dgrp + 1) * 512)
                            x2 = fp.tile([P, 4 * P], F32, name="f_x2",
                                         tag="f_x2")
                            nc.vector.tensor_add(x2[:], x1[tch][:, sl],
                                                 dps[tch * 4 + dgrp][:])
                            dlt = fp.tile([P, 4 * P], F32, name="f_dlt",
                                          tag="f_dlt")
                            nc.vector.tensor_sub(dlt[:], x2[:],
                                                 selh[tch][:, sl])
                            nc.vector.scalar_tensor_tensor(
                                upd[:, sl], in0=dlt[:],
                                scalar=gate_g[tch][:, :1],
                                in1=selh[tch][:, sl], op0=OP.mult, op1=OP.add)
                        nc.sync.dma_start(
                            out=upd_out[tch * P:(tch + 1) * P, :],
                            in_=upd[:])
                        nc.sync.dma_start(
                            out=selidx_out[tch * P:(tch + 1) * P, :],
                            in_=own_rows[tch][:])
                    nc.vector.tensor_copy(dbg_t[:, 8:9], gate_g[0][:])
                    nc.sync.dma_start(out=dbg[:], in_=dbg_t[:])
        else:
            with tc.tile_pool(name="rfin", bufs=2) as fp:
                for half in range(2):
                    nc.sync.dma_start(
                        out=upd_out[half * P:(half + 1) * P, :],
                        in_=selh[half][:])
                    nc.sync.dma_start(
                        out=selidx_out[half * P:(half + 1) * P, :],
                        in_=own_rows[half][:])
                nc.vector.tensor_copy(dbg_t[:, 4:5], p_cols[0][:])
                nc.vector.tensor_copy(dbg_t[:, 5:6], gate_g[0][:])
                nc.sync.dma_start(out=dbg[:], in_=dbg_t[:])


def _row_select_bcast(nc, pool, src_all, col_b, out_bcast):
    """out = broadcast(src_all row-block b), b in {0,1} from col_b."""
    r0 = pool.tile([1, T], F32, name="rs_r0")
    r1 = pool.tile([1, T], F32, name="rs_r1")
    v = src_all.rearrange("(a t) one -> a (t one)", a=2)
    nc.sync.dma_start(out=r0[:], in_=v[0:1, :])
    nc.sync.dma_start(out=r1[:], in_=v[1:2, :])
    b0 = pool.tile([P, T], F32, name="rs_b0")
    b1 = pool.tile([P, T], F32, name="rs_b1")
    nc.gpsimd.partition_broadcast(b0[:], r0[:])
    nc.gpsimd.partition_broadcast(b1[:], r1[:])
    df = pool.tile([P, T], F32, name="rs_df")
    nc.vector.tensor_sub(df[:], b1[:], b0[:])
    nc.vector.scalar_tensor_tensor(out_bcast[:], in0=df[:], scalar=col_b,
                                   in1=b0[:], op0=OP.mult, op1=OP.add)


# =====================================================================
# Host side
# =====================================================================
def kernel(**inputs):
    bf = ml_dtypes.bfloat16
    hs = np.asarray(inputs["hidden_states"], np.float32)
    qw = np.ascontiguousarray(np.asarray(inputs["q_w"], np.float32).astype(bf))
    kw = np.ascontiguousarray(np.asarray(inputs["k_w"], np.float32).astype(bf))
    vw = np.ascontiguousarray(np.asarray(inputs["v_w"], np.float32).astype(bf))
    bcu = float(np.asarray(inputs["beta_cu"]))
    bce = float(np.asarray(inputs["beta_ce"]))
    ceo = float(np.asarray(inputs["ce_off"]))

    hs_f = np.ascontiguousarray(hs.reshape(BT, D))
    orig_f = np.asarray(inputs["original"], np.float32).reshape(BT, D)
    post_f = np.asarray(inputs["posterior"], np.float32).reshape(BT, D)
    prior_f = np.asarray(inputs["prior"], np.float32).reshape(BT, D)
    cos_f = np.ascontiguousarray(
        np.asarray(inputs["cos"], np.float32).reshape(BT, HD))
    sin_f = np.ascontiguousarray(
        np.asarray(inputs["sin"], np.float32).reshape(BT, HD))

    gw_b = np.ascontiguousarray(
        np.asarray(inputs["gate_w"], np.float32).astype(bf))
    uw_b = np.ascontiguousarray(
        np.asarray(inputs["up_w"], np.float32).astype(bf))
    dw_b = np.ascontiguousarray(
        np.asarray(inputs["down_w"], np.float32).astype(bf))

    in_maps = []
    for c in range(NC):
        sl = slice(c * TOKS, (c + 1) * TOKS)
        b = c // 4
        cconst = np.array([[bcu, bce, bce * ceo, c * SB, 0.0,
                            0.0, (c % 4) * TOKS, b]], np.float32)
        in_maps.append({
            "orig_s": np.ascontiguousarray(orig_f[sl]),
            "post_s": np.ascontiguousarray(post_f[sl]),
            "prior_s": np.ascontiguousarray(prior_f[sl]),
            "hidden": hs_f,
            "cosf": cos_f,
            "sinf": sin_f,
            "qw": qw,
            "kw": kw,
            "vw": vw,
            "qb": np.asarray(inputs["q_b"], np.float32).reshape(-1, 1),
            "kb": np.asarray(inputs["k_b"], np.float32).reshape(-1, 1),
            "vb": np.asarray(inputs["v_b"], np.float32).reshape(-1, 1),
            "ow": np.asarray(inputs["o_w"], np.float32),
            "ln1w": np.asarray(inputs["ln1_w"], np.float32).reshape(-1, 1),
            "ln2w": np.asarray(inputs["ln2_w"], np.float32).reshape(-1, 1),
            "gatew": gw_b,
            "upw": uw_b,
            "downw": dw_b,
            "cconst": cconst,
        })

    global _last_in_maps
    _last_in_maps = in_maps
    import os
    ph = os.environ.get("KPHASES", "full")
    if ph not in _NC_CACHE:
        _NC_CACHE[ph] = build(phases=ph)
    nc = _NC_CACHE[ph]
    res = run_bass_kernel_spmd(nc, in_maps, core_ids=list(range(NC)))

    global _last_results
    _last_results = [res.results[c] for c in range(NC)]
    out = hs_f.copy()
    for c in range(NC):
        idx = res.results[c]["selidx_out"][:, 0]
        out[idx] = res.results[c]["upd_out"]
    return out.reshape(B, T, D)


if __name__ == "__main__":
    import reference
    inp = {k: np.asarray(v) for k, v in reference.setup_inputs().items()}
    got = kernel(**inp)
    want = np.asarray(reference.reference(**reference.setup_inputs()))
    err = np.abs(got - want).max() / np.abs(want).max()
    print("rel err:", err)


# revision 32
# speedup vs baseline: 1.0097x; 1.0097x over previous
"""Trainium2 Bass kernel for nn_DTFDynamicLayer (moe_routing dynamic-token
transformer layer), SPMD across 8 NeuronCores.

kernel(**inputs) takes FULL unsharded numpy inputs (keys as in setup_inputs)
and returns the FULL [B,T,D] output. Sharding (v2, data-parallel):
  - router (scores/topk/positions): token-sharded + tiny AllGathers
  - packed sequence S=2048 split in 8 contiguous blocks of 256 (one per core)
  - Q/K/V projections, RoPE: data-parallel (each core: all 16 heads for its
    own 256 packed tokens), weights in bf16
  - ONE combined AllGather of bf16 K^T+V (4096x256 per core) for attention
  - attention (all heads, own 256 queries over full S), O-proj: local
  - MLP: tensor-parallel over intermediate dim (704/core) over full S in
    bf16, partials combined with f32 ReduceScatter back to own positions
"""
from contextlib import ExitStack

import numpy as np
import ml_dtypes

import concourse.bass as bass
import concourse.mybir as mybir
import concourse.tile as tile
from concourse import bacc
from concourse.bass_utils import run_bass_kernel_spmd
from concourse.masks import make_identity

B, T, D = 2, 2048, 2048
H, HD = 16, 128
I = 5632
EPS = 1e-6
NC = 8
BT = B * T
TOKS = BT // NC          # 512 router tokens per core
K = T // 2               # 1024 selected per batch row
S = B * K                # 2048 packed tokens
SB = S // NC             # 256 packed slots per core
ICOL = I // NC           # 704
DC = D // 128            # 16
SCALE = 1.0 / float(np.sqrt(HD))
IC_CH = [128] * 5 + [ICOL - 5 * 128]   # I-col chunks per core: 5x128 + 64

F32 = mybir.dt.float32
F32R = mybir.dt.float32r
BF16 = mybir.dt.bfloat16
I32 = mybir.dt.int32
AF = mybir.ActivationFunctionType
OP = mybir.AluOpType
P = 128

_NC_CACHE = {}


def _rmsnorm_now(nc, pool, x, out, epst):
    """out = x * rsqrt(mean(x^2)+eps)  ([128, D] token-major, no weight)."""
    sq = pool.tile([P, D], F32, name="rn_sq")
    ssq = pool.tile([P, 1], F32, name="rn_ssq")
    nc.scalar.activation(sq[:], x[:], AF.Square, accum_out=ssq[:])
    rt = pool.tile([P, 1], F32, name="rn_rt")
    nc.scalar.activation(rt[:], ssq[:], AF.Sqrt, scale=1.0 / D,
                         bias=epst[:, :1])
    rec = pool.tile([P, 1], F32, name="rn_rec")
    nc.vector.reciprocal(rec[:], rt[:])
    nc.scalar.activation(out[:], x[:], AF.Copy, scale=rec[:, :1])


def _rope(nc, pool, q, cosT, sinm, out_ap, width):
    """q [128(hd), width] one head, feature-major. out = q*cos + rot(q)*sinm.
    rot(q)[0:64]=q[64:128], rot(q)[64:128]=q[0:64]; sinm rows 0:64 pre-negated.
    out_ap dtype may differ (e.g. bf16) - converted on the final add."""
    rot = pool.tile([P, width], F32, name="rp_rot", tag="rp_rot")
    nc.vector.tensor_copy(rot[0:64, :], q[64:P, :])
    nc.vector.tensor_copy(rot[64:P, :], q[0:64, :])
    t1 = pool.tile([P, width], F32, name="rp_t1", tag="rp_t1")
    nc.vector.tensor_mul(t1[:], q[:], cosT[:, :width])
    t2 = pool.tile([P, width], F32, name="rp_t2", tag="rp_t2")
    nc.vector.tensor_mul(t2[:], rot[:], sinm[:, :width])
    nc.vector.tensor_add(out_ap, t1[:], t2[:])


def _gather_cossin_T(nc, pool, ppool, ident, cosf, sinf, rows_col, cosT, sinm,
                     col_off):
    """Gather cos/sin rows (128 of them, by rows_col int32 [128,1]) and write
    transposed into cosT/sinm at column offset col_off. sinm rows 0:64 negated.
    """
    for (src, dstT, negate) in ((cosf, cosT, False), (sinf, sinm, True)):
        g = pool.tile([P, HD], F32, name="cs_g", tag="cs_g")
        nc.gpsimd.indirect_dma_start(
            out=g[:], out_offset=None, in_=src[:],
            in_offset=bass.IndirectOffsetOnAxis(ap=rows_col, axis=0))
        pt = ppool.tile([P, P], F32, space="PSUM", name="cs_p", tag="cs_p")
        nc.tensor.transpose(pt[:], g[:], ident[:])
        sl = slice(col_off, col_off + P)
        if negate:
            nc.scalar.activation(dstT[0:64, sl], pt[0:64, :], AF.Copy,
                                 scale=-1.0)
            nc.scalar.activation(dstT[64:P, sl], pt[64:P, :], AF.Copy)
        else:
            nc.vector.tensor_copy(dstT[:, sl], pt[:])


def build(phases="full"):
    nc = bacc.Bacc(None, target_bir_lowering=False)
    _build(nc, phases)
    nc.finalize()
    return nc


def _build(nc, phases):
    dp = nc.declare_dram_parameter
    orig_s = dp("orig_s", [TOKS, D], F32, isOutput=False)
    post_s = dp("post_s", [TOKS, D], F32, isOutput=False)
    prior_s = dp("prior_s", [TOKS, D], F32, isOutput=False)
    hidden = dp("hidden", [BT, D], F32, isOutput=False)
    cosf = dp("cosf", [BT, HD], F32, isOutput=False)
    sinf = dp("sinf", [BT, HD], F32, isOutput=False)
    qw = dp("qw", [D, H * HD], BF16, isOutput=False)
    kw = dp("kw", [D, H * HD], BF16, isOutput=False)
    vw = dp("vw", [D, H * HD], BF16, isOutput=False)
    qb = dp("qb", [H * HD, 1], F32, isOutput=False)
    kb = dp("kb", [H * HD, 1], F32, isOutput=False)
    vb = dp("vb", [H * HD, 1], F32, isOutput=False)
    ow = dp("ow", [H * HD, D], F32, isOutput=False)
    ln1w = dp("ln1w", [D, 1], F32, isOutput=False)
    ln2w = dp("ln2w", [D, 1], F32, isOutput=False)
    gatew = dp("gatew", [D, I], BF16, isOutput=False)
    upw = dp("upw", [D, I], BF16, isOutput=False)
    downw = dp("downw", [I, D], BF16, isOutput=False)
    # cconst: [beta_cu, beta_ce, beta_ce*ce_off, i0(=c*SB), unused,
    #          unused, i0row(=(c%4)*TOKS), b(=c//4)]
    cconst = dp("cconst", [1, 8], F32, isOutput=False)

    upd_out = dp("upd_out", [SB, D], F32, isOutput=True)
    selidx_out = dp("selidx_out", [SB, 1], I32, isOutput=True)
    dbg = dp("dbg", [P, 16], F32, isOutput=True)

    RG = [list(range(NC))]

    with tile.TileContext(nc) as tc, ExitStack() as es:
        # -------- DRAM internals (pool tiles => dep tracking) --------
        dr = es.enter_context(tc.tile_pool(name="dram", bufs=1, space="DRAM"))

        def dtile(name, shape, dtype=F32, shared=False):
            return dr.tile(shape, dtype, name=name,
                           addr_space="Shared" if shared else "Local")

        sc_in = dtile("sc_in", [TOKS, 1])
        sc_all = dtile("sc_all", [BT, 1], shared=True)
        mk_in = dtile("mk_in", [TOKS, 1])
        mk_all = dtile("mk_all", [BT, 1], shared=True)
        ps_in = dtile("ps_in", [TOKS, 1])
        ps_all = dtile("ps_all", [BT, 1], shared=True)
        selidx_d = dtile("selidx_d", [S + P, 1], I32)
        # kv_in rows 0..2047: K^T own (row h*128+d, col own token)
        # rows 2048..4095: V own [256 tok, 2048 hd] viewed as [2048, 256]
        kv_in = dtile("kv_in", [2 * H * HD, SB], BF16)
        kv_all = dtile("kv_all", [NC * 2 * H * HD, SB], BF16, shared=True)

        # -------- persistent SBUF --------
        pers = es.enter_context(tc.tile_pool(name="pers", bufs=1))
        ident = pers.tile([P, P], F32)
        make_identity(nc, ident[:])
        cc_sb = pers.tile([1, 8], F32)
        nc.sync.dma_start(out=cc_sb[:], in_=cconst[:])
        ccb = pers.tile([P, 8], F32)
        nc.gpsimd.partition_broadcast(ccb[:], cc_sb[:])
        col_bcu = ccb[:, 0:1]
        col_bce = ccb[:, 1:2]
        col_ceo = ccb[:, 2:3]
        col_i0 = ccb[:, 3:4]
        col_i0row = ccb[:, 6:7]
        col_b = ccb[:, 7:8]
        epst = pers.tile([P, 1], F32)
        nc.vector.memset(epst[:], EPS)
        iota_pf = pers.tile([P, SB], F32)      # value = p - f
        _it = pers.tile([P, SB], I32)
        nc.gpsimd.iota(_it[:], pattern=[[-1, SB]], base=0, channel_multiplier=1)
        nc.vector.tensor_copy(iota_pf[:], _it[:])
        iota_jmp = pers.tile([P, T], F32)      # value = j - p
        _it2 = pers.tile([P, T], I32)
        nc.gpsimd.iota(_it2[:], pattern=[[1, T]], base=0, channel_multiplier=-1)
        nc.vector.tensor_copy(iota_jmp[:], _it2[:])
        lnw_cols = pers.tile([P, 2 * DC], F32)  # [:, 0:16]=ln1, [:,16:32]=ln2
        nc.sync.dma_start(out=lnw_cols[:, 0:DC],
                          in_=ln1w.rearrange("(d p) one -> p d one", p=P))
        nc.sync.dma_start(out=lnw_cols[:, DC:2 * DC],
                          in_=ln2w.rearrange("(d p) one -> p d one", p=P))
        dbg_t = pers.tile([P, 16], F32)
        nc.vector.memset(dbg_t[:], 0.0)

        s_cols = [pers.tile([P, 1], F32, name=f"s_col{t}") for t in range(4)]
        m_cols = [pers.tile([P, 1], F32, name=f"m_col{t}") for t in range(4)]
        p_cols = [pers.tile([P, 1], F32, name=f"p_col{t}") for t in range(4)]

        # ============ Phase R1: scores for own 512 tokens ============
        with tc.tile_pool(name="router", bufs=2) as rp:
            for t in range(4):
                cu = rp.tile([P, 1], F32, name="cu")
                ce = rp.tile([P, 1], F32, name="ce")
                tsl = slice(t * P, (t + 1) * P)
                ot = rp.tile([P, D], F32, name="r_ot")
                pt_ = rp.tile([P, D], F32, name="r_pt")
                rt_ = rp.tile([P, D], F32, name="r_rt")
                nc.sync.dma_start(out=ot[:], in_=orig_s[tsl, :])
                nc.sync.dma_start(out=pt_[:], in_=post_s[tsl, :])
                nc.sync.dma_start(out=rt_[:], in_=prior_s[tsl, :])
                for (a_t, b_t, dst) in ((ot, pt_, cu), (pt_, rt_, ce)):
                    df = rp.tile([P, D], F32, name="r_df")
                    nc.vector.tensor_sub(df[:], a_t[:], b_t[:])
                    sq = rp.tile([P, D], F32, name="r_sq")
                    ssq = rp.tile([P, 1], F32, name="r_ssq")
                    nc.scalar.activation(sq[:], df[:], AF.Square,
                                         accum_out=ssq[:])
                    nc.scalar.activation(dst[:], ssq[:], AF.Sqrt)
                t1 = rp.tile([P, 1], F32, name="r_t1")
                nc.vector.tensor_scalar(t1[:], cu[:], col_bcu, None,
                                        op0=OP.mult)
                nc.vector.scalar_tensor_tensor(
                    s_cols[t][:], in0=ce[:], scalar=col_bce, in1=t1[:],
                    op0=OP.mult, op1=OP.add)
                nc.vector.tensor_scalar(s_cols[t][:], s_cols[t][:], col_ceo,
                                        None, op0=OP.add)
            sc_flat = rp.tile([P, 4], F32, name="scflat")
            for t in range(4):
                nc.vector.tensor_copy(sc_flat[:, t:t + 1], s_cols[t][:])
            nc.sync.dma_start(
                out=sc_in.rearrange("(t p) one -> p t one", p=P),
                in_=sc_flat[:])
        nc.gpsimd.collective_compute("AllGather", OP.bypass, replica_groups=RG,
                                     ins=[sc_in[:]], outs=[sc_all[:]])

        if phases == "score":
            with tc.tile_pool(name="sfin", bufs=1) as fp:
                sall = fp.tile([P, BT // P], F32, name="sall")
                nc.sync.dma_start(
                    out=sall[:],
                    in_=sc_all.rearrange("(t p) one -> p t one", p=P))
                nc.vector.tensor_copy(dbg_t[:, 0:1], sall[:, 0:1])
                nc.vector.tensor_copy(dbg_t[:, 1:2], sall[:, 31:32])
                nc.vector.tensor_copy(dbg_t[:, 2:3], s_cols[0][:])
                nc.sync.dma_start(out=dbg[:], in_=dbg_t[:])
            return

        # ============ Phase R2: rank -> mask for own tokens ============
        # rank_i = #{j: s_j>s_i} + #{j<i: s_j==s_i} = (T - sum(le)) + sum(eq*jlt)
        # mask = rank <= K-1  <=>  acc = sum(le) - sum(eq*jlt) >= T-K+1
        with tc.tile_pool(name="rank", bufs=2) as rp:
            sbr = rp.tile([P, T], F32, name="sbr")
            _row_select_bcast(nc, rp, sc_all, col_b, sbr)
            for t in range(4):
                # no-tie rank: rank_i = T - sum(le); random f32 scores make
                # exact duplicates measure-zero, so tie-break terms dropped
                le = rp.tile([P, T], F32, name="k_le")
                nc.vector.tensor_scalar(le[:], sbr[:], s_cols[t][:, :1], None,
                                        op0=OP.is_le)
                acc = rp.tile([P, 1], F32, name="k_acc")
                nc.vector.tensor_reduce(acc[:], le[:],
                                        axis=mybir.AxisListType.X, op=OP.add)
                # mask = acc >= T-K+1  <=>  (-acc) <= -(T-K+1)
                nacc = rp.tile([P, 1], F32, name="k_nacc")
                nc.vector.tensor_scalar_mul(nacc[:], acc[:], -1.0)
                nc.vector.tensor_scalar(m_cols[t][:], nacc[:],
                                        float(-(T - K + 1)), None,
                                        op0=OP.is_le)
                if t == 0:
                    nc.vector.tensor_copy(dbg_t[:, 0:1], acc[:])
                    nc.vector.tensor_copy(dbg_t[:, 1:2], m_cols[t][:])
                    nc.vector.tensor_copy(dbg_t[:, 2:3], s_cols[t][:])
            mflat = rp.tile([P, 4], F32, name="mflat")
            for t in range(4):
                nc.vector.tensor_copy(mflat[:, t:t + 1], m_cols[t][:])
            nc.sync.dma_start(
                out=mk_in.rearrange("(t p) one -> p t one", p=P), in_=mflat[:])
        nc.gpsimd.collective_compute("AllGather", OP.bypass, replica_groups=RG,
                                     ins=[mk_in[:]], outs=[mk_all[:]])

        if phases == "rank":
            with tc.tile_pool(name="kfin", bufs=1) as fp:
                mall = fp.tile([P, BT // P], F32, name="mall")
                nc.sync.dma_start(
                    out=mall[:],
                    in_=mk_all.rearrange("(t p) one -> p t one", p=P))
                nc.vector.tensor_copy(dbg_t[:, 4:5], mall[:, 0:1])
                nc.vector.tensor_copy(dbg_t[:, 5:6], mall[:, 31:32])
                nc.sync.dma_start(out=dbg[:], in_=dbg_t[:])
            return

        # ============ Phase R3: positions ============
        with tc.tile_pool(name="pos", bufs=2) as rp:
            mbr = rp.tile([P, T], F32, name="mbr")
            _row_select_bcast(nc, rp, mk_all, col_b, mbr)
            for t in range(4):
                jlt = rp.tile([P, T], F32, name="p_jlt")
                rhs = rp.tile([P, 1], F32, name="p_rhs")
                nc.vector.tensor_scalar(rhs[:], col_i0row, float(t * P - 1),
                                        None, op0=OP.add)
                nc.vector.tensor_scalar(jlt[:], iota_jmp[:], rhs[:, :1], None,
                                        op0=OP.is_le)
                mj = rp.tile([P, T], F32, name="p_mj")
                nc.vector.tensor_mul(mj[:], mbr[:], jlt[:])
                nc.vector.tensor_reduce(p_cols[t][:], mj[:],
                                        axis=mybir.AxisListType.X, op=OP.add)
                if t == 0:
                    nc.vector.tensor_copy(dbg_t[:, 3:4], p_cols[t][:])
            pflat = rp.tile([P, 4], F32, name="pflat")
            for t in range(4):
                nc.vector.tensor_copy(pflat[:, t:t + 1], p_cols[t][:])
            nc.sync.dma_start(
                out=ps_in.rearrange("(t p) one -> p t one", p=P), in_=pflat[:])
        nc.gpsimd.collective_compute("AllGather", OP.bypass, replica_groups=RG,
                                     ins=[ps_in[:]], outs=[ps_all[:]])

        if phases == "pos":
            with tc.tile_pool(name="pfin", bufs=1) as fp:
                pall = fp.tile([P, BT // P], F32, name="pall")
                nc.sync.dma_start(
                    out=pall[:],
                    in_=ps_all.rearrange("(t p) one -> p t one", p=P))
                nc.vector.tensor_copy(dbg_t[:, 4:5], pall[:, 0:1])
                nc.vector.tensor_copy(dbg_t[:, 5:6], pall[:, 31:32])
                nc.sync.dma_start(out=dbg[:], in_=dbg_t[:])
            return

        # ============ Phase SCT: slot -> flat row map ============
        with tc.tile_pool(name="scat", bufs=4) as sp:
            zt = sp.tile([P, (S + P) // P], I32, name="sc_zero")
            nc.vector.memset(zt[:], 0)
            nc.sync.dma_start(
                out=selidx_d.rearrange("(t p) one -> p t one", p=P), in_=zt[:])
            mk_t = sp.tile([P, BT // P], F32, name="mk_t")
            ps_t = sp.tile([P, BT // P], F32, name="ps_t")
            nc.sync.dma_start(out=mk_t[:],
                              in_=mk_all.rearrange("(t p) one -> p t one", p=P))
            nc.sync.dma_start(out=ps_t[:],
                              in_=ps_all.rearrange("(t p) one -> p t one", p=P))
            dump_i = sp.tile([P, 1], I32, name="sc_dumpi")
            nc.gpsimd.iota(dump_i[:], pattern=[[0, 1]], base=S,
                           channel_multiplier=1)
            dump_f = sp.tile([P, 1], F32, name="sc_dumpf")
            nc.vector.tensor_copy(dump_f[:], dump_i[:])
            dump_ni = sp.tile([P, 1], I32, name="sc_dumpni")
            nc.gpsimd.iota(dump_ni[:], pattern=[[0, 1]], base=-S,
                           channel_multiplier=-1)
            dump_nf = sp.tile([P, 1], F32, name="sc_dumpnf")
            nc.vector.tensor_copy(dump_nf[:], dump_ni[:])
            # batched slot computation over all 32 chunks:
            # slot' = m*(pos + b*K - (S+p)) + (S+p)  (per-part dump row)
            NCH = BT // P
            t1 = sp.tile([P, NCH], F32, name="sc_t1")
            for b in range(2):
                hsl = slice(b * (NCH // 2), (b + 1) * (NCH // 2))
                nc.vector.tensor_scalar(t1[:, hsl], ps_t[:, hsl],
                                        float(b * K), None, op0=OP.add)
            nc.vector.tensor_scalar(t1[:], t1[:], dump_nf[:, :1], None,
                                    op0=OP.add)
            t2 = sp.tile([P, NCH], F32, name="sc_t2")
            nc.vector.tensor_mul(t2[:], t1[:], mk_t[:])
            nc.vector.tensor_scalar(t2[:], t2[:], dump_f[:, :1], None,
                                    op0=OP.add)
            off_i = sp.tile([P, NCH], I32, name="sc_off")
            nc.vector.tensor_copy(off_i[:], t2[:])
            val_i = sp.tile([P, NCH], I32, name="sc_val")
            nc.gpsimd.iota(val_i[:], pattern=[[P, NCH]], base=0,
                           channel_multiplier=1)
            for t in range(NCH):
                nc.gpsimd.indirect_dma_start(
                    out=selidx_d[:],
                    out_offset=bass.IndirectOffsetOnAxis(ap=off_i[:, t:t + 1],
                                                         axis=0),
                    in_=val_i[:, t:t + 1], in_offset=None)

        # ============ Phase G: gathers ============
        gpL = es.enter_context(tc.tile_pool(name="gpL", bufs=1))   # long-lived
        own_rows = []
        selh = []
        gate_g = []
        myslot = gpL.tile([P, 2], I32)
        _si = gpL.tile([P, 2], I32)
        _slotf = gpL.tile([P, 2], F32)
        for half in range(2):
            nc.gpsimd.iota(_si[:, half:half + 1], pattern=[[0, 1]],
                           base=half * P, channel_multiplier=1)
        nc.vector.tensor_copy(_slotf[:], _si[:])
        for half in range(2):
            nc.vector.tensor_scalar(_slotf[:, half:half + 1],
                                    _slotf[:, half:half + 1], col_i0, None,
                                    op0=OP.add)
        nc.vector.tensor_copy(myslot[:], _slotf[:])
        for half in range(2):
            orow = gpL.tile([P, 1], I32, name=f"orow{half}")
            nc.gpsimd.indirect_dma_start(
                out=orow[:], out_offset=None, in_=selidx_d[:],
                in_offset=bass.IndirectOffsetOnAxis(
                    ap=myslot[:, half:half + 1], axis=0))
            own_rows.append(orow)
            sh = gpL.tile([P, D], F32, name=f"selh{half}")
            nc.gpsimd.indirect_dma_start(
                out=sh[:], out_offset=None, in_=hidden[:],
                in_offset=bass.IndirectOffsetOnAxis(ap=orow[:, :1], axis=0),
                bounds_check=BT - 1, oob_is_err=False)
            selh.append(sh)
            ssc = gpL.tile([P, 1], F32, name=f"ssc{half}")
            nc.gpsimd.indirect_dma_start(
                out=ssc[:], out_offset=None, in_=sc_all[:],
                in_offset=bass.IndirectOffsetOnAxis(ap=orow[:, :1], axis=0))
            gg = gpL.tile([P, 1], F32, name=f"gate{half}")
            nc.scalar.activation(gg[:], ssc[:], AF.Sigmoid)
            gate_g.append(gg)
        x1 = [gpL.tile([P, D], F32, name=f"x1_{i}") for i in range(2)]

        if phases == "full":
            # attention-lived pool (opened before gpQ: LIFO close order)
            esA = ExitStack()
            gpA = esA.enter_context(tc.tile_pool(name="gpA", bufs=1))
            q_own = gpA.tile([P, H, SB], BF16)
            o_fm = gpA.tile([P, H, SB], F32R)

            # mid-lived pool: through QKV
            esQ = ExitStack()
            gpQ = esQ.enter_context(tc.tile_pool(name="gpQ", bufs=1))
            cosT_o = gpQ.tile([P, SB], F32)
            sinm_o = gpQ.tile([P, SB], F32)
            h1T_own = gpQ.tile([P, DC, SB], BF16)
            vt_blk = gpQ.tile([P, 2, H * HD], BF16)
            with tc.tile_pool(name="cso", bufs=3) as cp, \
                 tc.tile_pool(name="csop", bufs=4, space="PSUM") as cpp:
                for half in range(2):
                    _gather_cossin_T(nc, cp, cpp, ident, cosf, sinf,
                                     own_rows[half][:, :1], cosT_o, sinm_o,
                                     half * P)

            # ============ Phase N1: h1 = rmsnorm(selh); h1T own (bf16) ======
            with tc.tile_pool(name="n1", bufs=2) as np_, \
                 tc.tile_pool(name="n1p", bufs=4, space="PSUM") as npp:
                for half in range(2):
                    h1 = np_.tile([P, D], F32, name="h1")
                    _rmsnorm_now(nc, np_, selh[half], h1, epst)
                    for d in range(DC):
                        pt = npp.tile([P, P], F32, space="PSUM", name="n1_tp")
                        nc.tensor.transpose(pt[:], h1[:, d * P:(d + 1) * P],
                                            ident[:])
                        # fold ln1 weight (per-d-row) into the PSUM->SBUF copy
                        nc.vector.tensor_scalar(
                            h1T_own[:, d, half * P:(half + 1) * P],
                            pt[:], lnw_cols[:, d:d + 1], None, op0=OP.mult)

            # ============ Phase QKV (DP: all heads, own 256 tokens) ========
            with tc.tile_pool(name="qkv", bufs=2) as qp, \
                 tc.tile_pool(name="qkvw", bufs=1) as wpool:

                def _proj(wap, b_ap, w_i, qpp, qtp):
                    wt = wpool.tile([P, DC, H * HD], BF16, name="wt",
                                    tag="wt")
                    nc.sync.dma_start(
                        out=wt[:], in_=wap.rearrange("(d p) c -> p d c", p=P))
                    bias = qp.tile([P, H], F32, name="bias", tag="bias")
                    nc.sync.dma_start(
                        out=bias[:],
                        in_=b_ap.rearrange("(h p) one -> p h one", p=P))
                    for g4 in range(4):
                        ps4 = [qpp.tile([P, SB], F32, space="PSUM",
                                        name=f"qkv_ps{j}", tag=f"qkv_ps{j}")
                               for j in range(4)]
                        for d in range(DC):
                            for j in range(4):
                                oc = g4 * 4 + j
                                nc.tensor.matmul(
                                    ps4[j][:], wt[:, d, oc * P:(oc + 1) * P],
                                    h1T_own[:, d, :], start=(d == 0),
                                    stop=(d == DC - 1))
                        for j in range(4):
                            oc = g4 * 4 + j
                            hq = qp.tile([P, SB], F32, name="qkv_h",
                                         tag="qkv_h")
                            nc.scalar.activation(hq[:], ps4[j][:], AF.Identity,
                                                 bias=bias[:, oc:oc + 1])
                            if w_i == 0:
                                _rope(nc, qp, hq, cosT_o, sinm_o,
                                      q_own[:, oc, :], SB)
                            elif w_i == 1:
                                kr = qp.tile([P, SB], BF16, name="kr",
                                             tag="kr")
                                _rope(nc, qp, hq, cosT_o, sinm_o, kr[:], SB)
                                nc.sync.dma_start(
                                    out=kv_in[oc * P:(oc + 1) * P, :],
                                    in_=kr[:])
                            else:
                                for half in range(2):
                                    ptt = qtp.tile([P, P], F32, space="PSUM",
                                                   name="v_tp", tag="v_tp")
                                    nc.tensor.transpose(
                                        ptt[:], hq[:, half * P:(half + 1) * P],
                                        ident[:])
                                    nc.vector.tensor_copy(
                                        vt_blk[:, half, oc * P:(oc + 1) * P],
                                        ptt[:])

                with tc.tile_pool(name="qkvp2", bufs=2, space="PSUM") as qpp2:
                    _proj(kw, kb, 1, qpp2, None)
                with tc.tile_pool(name="qkvp1", bufs=1, space="PSUM") as qpp1, \
                     tc.tile_pool(name="qkvtp", bufs=4, space="PSUM") as qtp:
                    _proj(vw, vb, 2, qpp1, qtp)
                for half in range(2):
                    nc.sync.dma_start(
                        out=kv_in[2048 + half * 1024:
                                  2048 + (half + 1) * 1024, :].rearrange(
                                      "(p g) c -> p (g c)", p=P),
                        in_=vt_blk[:, half, :])
                # K/V shipped; Q-proj below overlaps the AllGather
                nc.gpsimd.collective_compute("AllGather", OP.bypass,
                                             replica_groups=RG,
                                             ins=[kv_in[:]], outs=[kv_all[:]])
                with tc.tile_pool(name="qkvp3", bufs=2, space="PSUM") as qpp3:
                    _proj(qw, qb, 0, qpp3, None)
            esQ.close()

            # OPROJ weight prefetch: first 2 groups load during the AG/ATT
            # window (DMA engines are otherwise idle during the collective)
            op2 = esA.enter_context(tc.tile_pool(name="opj2", bufs=2))
            ow_view = ow.rearrange("(x p) c -> p x c", p=P)
            ow_tiles = {}
            for grp in range(2):
                og = op2.tile([P, H, 4 * P], F32R, name="ow_g", tag="ow_g")
                nc.sync.dma_start(
                    out=og[:],
                    in_=ow_view[:, :, grp * 4 * P:(grp + 1) * 4 * P].bitcast(
                        F32R))
                ow_tiles[grp] = og

            # ============ Phase ATT ============
            with tc.tile_pool(name="att", bufs=2) as ap, \
                 tc.tile_pool(name="att1", bufs=1) as ap1, \
                 tc.tile_pool(name="attp", bufs=2, space="PSUM") as app:
                # causal masks per j-chunk: keep (p - f) <= i0 - jc*128
                # (no dep on kv_all - scheduler overlaps this with the AG)
                masks = ap1.tile([P, S // P, SB], BF16, name="masks")
                for jc in range(S // P):
                    rhsc = ap.tile([P, 1], F32, name="a_rhs")
                    nc.vector.tensor_scalar(rhsc[:], col_i0, float(-jc * P),
                                            None, op0=OP.add)
                    nc.vector.tensor_scalar(masks[:, jc, :], iota_pf[:],
                                            rhsc[:, :1], None, op0=OP.is_le)
                onesb = ap1.tile([P, 1], BF16, name="onesb")
                nc.vector.memset(onesb[:], 1.0)
                kv_view = kv_all.rearrange("(c x) s -> x c s", c=NC)
                vv_view = kv_all.rearrange(
                    "(c z tc p g) s -> z g p c tc s",
                    c=NC, z=2, tc=2, p=P, g=8)
                for h in range(H):
                    khead = ap.tile([P, NC, SB], BF16, name="khead")
                    nc.sync.dma_start(
                        out=khead[:], in_=kv_view[h * P:(h + 1) * P, :, :])
                    if h % 2 == 0:
                        # one load covers this head pair (512B elems, no
                        # sub-512B DMA penalty)
                        vpair = ap.tile([P, 2, NC, 2 * P], BF16, name="vpair",
                                        tag="vpair")
                        for tcv in range(2):
                            nc.sync.dma_start(
                                out=vpair[:, tcv, :, :],
                                in_=vv_view[1, h // 2, :, :, tcv, :])
                    hc0 = (h % 2) * P
                    psum_o = app.tile([P, SB], F32, space="PSUM", name="a_po")
                    psum_s = app.tile([1, SB], F32, space="PSUM", name="a_ps")
                    for jp in range(S // P // 2):
                        # paired k-chunks: 2 QK matmuls -> one 512-wide
                        # exp+mask (Act is the steady-state bottleneck)
                        pa2 = app.tile([P, 2 * SB], F32, space="PSUM",
                                       name="a_pa")
                        for u in range(2):
                            jc = 2 * jp + u
                            nc.tensor.matmul(
                                pa2[:, u * SB:(u + 1) * SB],
                                khead[:, jc // 2,
                                      (jc % 2) * P:(jc % 2 + 1) * P],
                                q_own[:, h, :], start=True, stop=True,
                                skip_group_check=True)
                        et = ap.tile([P, 2 * SB], BF16, name="a_et")
                        nc.scalar.activation(et[:], pa2[:], AF.Exp,
                                             scale=SCALE)
                        ex = ap.tile([P, 2 * SB], BF16, name="a_ex")
                        nc.vector.tensor_mul(ex[:], et[:],
                                             masks[:, 2 * jp:2 * jp + 2, :])
                        for u in range(2):
                            jc = 2 * jp + u
                            exs = ex[:, u * SB:(u + 1) * SB]
                            nc.tensor.matmul(psum_s[:], onesb[:], exs,
                                             start=(jc == 0),
                                             stop=(jc == S // P - 1),
                                             skip_group_check=True)
                            nc.tensor.matmul(psum_o[:],
                                             vpair[:, jc % 2, jc // 2,
                                                   hc0:hc0 + P],
                                             exs,
                                             start=(jc == 0),
                                             stop=(jc == S // P - 1),
                                             skip_group_check=True)
                    rec = ap.tile([1, SB], F32, name="a_rec")
                    nc.vector.reciprocal(rec[:], psum_s[:])
                    recb = ap.tile([P, SB], F32, name="a_recb")
                    nc.gpsimd.partition_broadcast(recb[:], rec[:])
                    nc.vector.tensor_mul(o_fm[:, h, :], psum_o[:].bitcast(F32R),
                                         recb[:].bitcast(F32R))

            # ============ Phase OPROJ: x1 = selh + ow.T @ o_fm ============
            with tc.tile_pool(name="opj", bufs=1) as op_, \
                 tc.tile_pool(name="opjp", bufs=3, space="PSUM") as opp:
                for grp in range(4):      # D col groups of 512
                    if grp in ow_tiles:
                        ow_g = ow_tiles[grp]
                    else:
                        ow_g = op2.tile([P, H, 4 * P], F32R, name="ow_g",
                                        tag="ow_g")
                        nc.sync.dma_start(
                            out=ow_g[:],
                            in_=ow_view[:, :,
                                        grp * 4 * P:(grp + 1) * 4 * P].bitcast(
                                            F32R))
                    for dd in range(4):
                        d = grp * 4 + dd
                        pt = opp.tile([P, SB], F32, space="PSUM", name="o_ps")
                        for h in range(H):
                            nc.tensor.matmul(pt[:],
                                             ow_g[:, h, dd * P:(dd + 1) * P],
                                             o_fm[:, h, :], start=(h == 0),
                                             stop=(h == H - 1))
                        for half in range(2):
                            ot = op_.tile([P, P], F32, name="o_sb")
                            nc.vector.tensor_copy(ot[:],
                                                  pt[:, half * P:(half + 1) * P])
                            pt2 = opp.tile([P, P], F32, space="PSUM", name="o_tp")
                            nc.tensor.transpose(pt2[:], ot[:], ident[:])
                            nc.vector.tensor_add(
                                x1[half][:, d * P:(d + 1) * P], pt2[:],
                                selh[half][:, d * P:(d + 1) * P])
            esA.close()

            # ============ Phase N2 + MLP (DP: own 256 tokens, bf16) ========
            NIG = I // 512            # 11 i-groups of 512
            with tc.tile_pool(name="mlp", bufs=1) as mp, \
                 tc.tile_pool(name="mlps", bufs=1) as mps, \
                 tc.tile_pool(name="mlp2", bufs=2) as mp2:
                h2T_own = mp.tile([P, DC, SB], BF16, name="h2T_own")
                with tc.tile_pool(name="n2p", bufs=4, space="PSUM") as n2p:
                    for half in range(2):
                        h2 = mps.tile([P, D], F32, name="h2", tag="h2")
                        _rmsnorm_now(nc, mps, x1[half], h2, epst)
                        for d in range(DC):
                            pt = n2p.tile([P, P], F32, space="PSUM",
                                          name="m_tp")
                            nc.tensor.transpose(pt[:], h2[:, d * P:(d + 1) * P],
                                                ident[:])
                            nc.scalar.activation(
                                h2T_own[:, d, half * P:(half + 1) * P], pt[:],
                                AF.Copy, scale=lnw_cols[:, DC + d:DC + d + 1])
                act = mp.tile([P, I // P, SB], BF16, name="act")
                gw_view = gatew.rearrange("(d p) c -> p d c", p=P)
                uw_view = upw.rearrange("(d p) c -> p d c", p=P)
                with tc.tile_pool(name="mlpg", bufs=2) as mpg, \
                     tc.tile_pool(name="mlpp", bufs=2, space="PSUM") as mpp:
                    for ig in range(NIG):
                        isl = slice(ig * 512, (ig + 1) * 512)
                        gwt = mpg.tile([P, DC, 512], BF16, name="gwt",
                                       tag="gwt")
                        nc.sync.dma_start(out=gwt[:], in_=gw_view[:, :, isl])
                        uwt = mpg.tile([P, DC, 512], BF16, name="uwt",
                                       tag="uwt")
                        nc.sync.dma_start(out=uwt[:], in_=uw_view[:, :, isl])
                        for i4 in range(4):
                            ic = ig * 4 + i4
                            ptg = mpp.tile([P, SB], F32, space="PSUM",
                                           name="m_ptg", tag="m_ptg")
                            ptu = mpp.tile([P, SB], F32, space="PSUM",
                                           name="m_ptu", tag="m_ptu")
                            for d in range(DC):
                                nc.tensor.matmul(
                                    ptg[:], gwt[:, d, i4 * P:(i4 + 1) * P],
                                    h2T_own[:, d, :],
                                    start=(d == 0), stop=(d == DC - 1))
                            for d in range(DC):
                                nc.tensor.matmul(
                                    ptu[:], uwt[:, d, i4 * P:(i4 + 1) * P],
                                    h2T_own[:, d, :],
                                    start=(d == 0), stop=(d == DC - 1))
                            sg = mpg.tile([P, SB], F32, name="m_sg",
                                          tag="m_sg")
                            nc.scalar.activation(sg[:], ptg[:], AF.Silu)
                            nc.vector.tensor_mul(act[:, ic, :], sg[:], ptu[:])
                # down-proj (DP): accumulate own [256, D] across all 44 ic
                dw_view = downw.rearrange("(g p) c -> p g c", p=P)
                with tc.tile_pool(name="mlpd", bufs=1, space="PSUM") as dpp, \
                     tc.tile_pool(name="fin", bufs=2) as fp:
                    dps = [dpp.tile([P, 4 * P], F32, space="PSUM",
                                    name=f"dps{t}")
                           for t in range(8)]
                    for ig in range(NIG):
                        dwn = mp2.tile([P, 4, D], BF16, name="dwn", tag="dwn")
                        nc.sync.dma_start(
                            out=dwn[:],
                            in_=dw_view[:, ig * 4:(ig + 1) * 4, :])
                        for tch in range(2):
                            for dgrp in range(4):
                                ps = dps[tch * 4 + dgrp]
                                for i4 in range(4):
                                    ic = ig * 4 + i4
                                    nc.tensor.matmul(
                                        ps[:],
                                        act[:, ic, tch * P:(tch + 1) * P],
                                        dwn[:, i4,
                                            dgrp * 512:(dgrp + 1) * 512],
                                        start=(ig == 0 and i4 == 0),
                                        stop=(ig == NIG - 1 and i4 == 3),
                                        skip_group_check=True)
                    # ===== final gated update straight out of PSUM =====
                    for tch in range(2):
                        upd = fp.tile([P, D], F32, name="f_upd", tag="f_upd")
                        for dgrp in range(4):
                            sl = slice(dgrp * 512, (dgrp + 1) * 512)
                            x2 = fp.tile([P, 4 * P], F32, name="f_x2",
                                         tag="f_x2")
                            nc.vector.tensor_add(x2[:], x1[tch][:, sl],
                                                 dps[tch * 4 + dgrp][:])
                            dlt = fp.tile([P, 4 * P], F32, name="f_dlt",
                                          tag="f_dlt")
                            nc.vector.tensor_sub(dlt[:], x2[:],
                                                 selh[tch][:, sl])
                            nc.vector.scalar_tensor_tensor(
                                upd[:, sl], in0=dlt[:],
                                scalar=gate_g[tch][:, :1],
                                in1=selh[tch][:, sl], op0=OP.mult, op1=OP.add)
                        nc.sync.dma_start(
                            out=upd_out[tch * P:(tch + 1) * P, :],
                            in_=upd[:])
                        nc.sync.dma_start(
                            out=selidx_out[tch * P:(tch + 1) * P, :],
                            in_=own_rows[tch][:])
                    nc.vector.tensor_copy(dbg_t[:, 8:9], gate_g[0][:])
                    nc.sync.dma_start(out=dbg[:], in_=dbg_t[:])
        else:
            with tc.tile_pool(name="rfin", bufs=2) as fp:
                for half in range(2):
                    nc.sync.dma_start(
                        out=upd_out[half * P:(half + 1) * P, :],
                        in_=selh[half][:])
                    nc.sync.dma_start(
                        out=selidx_out[half * P:(half + 1) * P, :],
                        in_=own_rows[half][:])
                nc.vector.tensor_copy(dbg_t[:, 4:5], p_cols[0][:])
                nc.vector.tensor_copy(dbg_t[:, 5:6], gate_g[0][:])
                nc.sync.dma_start(out=dbg[:], in_=dbg_t[:])


def _row_select_bcast(nc, pool, src_all, col_b, out_bcast):
    """out = broadcast(src_all row-block b), b in {0,1} from col_b."""
    r0 = pool.tile([1, T], F32, name="rs_r0")
    r1 = pool.tile([1, T], F32, name="rs_r1")
    v = src_all.rearrange("(a t) one -> a (t one)", a=2)
    nc.sync.dma_start(out=r0[:], in_=v[0:1, :])
    nc.sync.dma_start(out=r1[:], in_=v[1:2, :])
    b0 = pool.tile([P, T], F32, name="rs_b0")
    b1 = pool.tile([P, T], F32, name="rs_b1")
    nc.gpsimd.partition_broadcast(b0[:], r0[:])
    nc.gpsimd.partition_broadcast(b1[:], r1[:])
    df = pool.tile([P, T], F32, name="rs_df")
    nc.vector.tensor_sub(df[:], b1[:], b0[:])
    nc.vector.scalar_tensor_tensor(out_bcast[:], in0=df[:], scalar=col_b,
                                   in1=b0[:], op0=OP.mult, op1=OP.add)


# =====================================================================
# Host side
# =====================================================================
def kernel(**inputs):
    bf = ml_dtypes.bfloat16
    hs = np.asarray(inputs["hidden_states"], np.float32)
    qw = np.ascontiguousarray(np.asarray(inputs["q_w"], np.float32).astype(bf))
    kw = np.ascontiguousarray(np.asarray(inputs["k_w"], np.float32).astype(bf))
    vw = np.ascontiguousarray(np.asarray(inputs["v_w"], np.float32).astype(bf))
    bcu = float(np.asarray(inputs["beta_cu"]))
    bce = float(np.asarray(inputs["beta_ce"]))
    ceo = float(np.asarray(inputs["ce_off"]))

    hs_f = np.ascontiguousarray(hs.reshape(BT, D))
    orig_f = np.asarray(inputs["original"], np.float32).reshape(BT, D)
    post_f = np.asarray(inputs["posterior"], np.float32).reshape(BT, D)
    prior_f = np.asarray(inputs["prior"], np.float32).reshape(BT, D)
    cos_f = np.ascontiguousarray(
        np.asarray(inputs["cos"], np.float32).reshape(BT, HD))
    sin_f = np.ascontiguousarray(
        np.asarray(inputs["sin"], np.float32).reshape(BT, HD))

    gw_b = np.ascontiguousarray(
        np.asarray(inputs["gate_w"], np.float32).astype(bf))
    uw_b = np.ascontiguousarray(
        np.asarray(inputs["up_w"], np.float32).astype(bf))
    dw_b = np.ascontiguousarray(
        np.asarray(inputs["down_w"], np.float32).astype(bf))

    in_maps = []
    for c in range(NC):
        sl = slice(c * TOKS, (c + 1) * TOKS)
        b = c // 4
        cconst = np.array([[bcu, bce, bce * ceo, c * SB, 0.0,
                            0.0, (c % 4) * TOKS, b]], np.float32)
        in_maps.append({
            "orig_s": np.ascontiguousarray(orig_f[sl]),
            "post_s": np.ascontiguousarray(post_f[sl]),
            "prior_s": np.ascontiguousarray(prior_f[sl]),
            "hidden": hs_f,
            "cosf": cos_f,
            "sinf": sin_f,
            "qw": qw,
            "kw": kw,
            "vw": vw,
            "qb": np.asarray(inputs["q_b"], np.float32).reshape(-1, 1),
            "kb": np.asarray(inputs["k_b"], np.float32).reshape(-1, 1),
            "vb": np.asarray(inputs["v_b"], np.float32).reshape(-1, 1),
            "ow": np.asarray(inputs["o_w"], np.float32),
            "ln1w": np.asarray(inputs["ln1_w"], np.float32).reshape(-1, 1),
            "ln2w": np.asarray(inputs["ln2_w"], np.float32).reshape(-1, 1),
            "gatew": gw_b,
            "upw": uw_b,
            "downw": dw_b,
            "cconst": cconst,
        })

    global _last_in_maps
    _last_in_maps = in_maps
    import os
    ph = os.environ.get("KPHASES", "full")
    if ph not in _NC_CACHE:
        _NC_CACHE[ph] = build(phases=ph)
    nc = _NC_CACHE[ph]
    res = run_bass_kernel_spmd(nc, in_maps, core_ids=list(range(NC)))

    global _last_results
    _last_results = [res.results[c] for c in range(NC)]
    out = hs_f.copy()
    for c in range(NC):
        idx = res.results[c]["selidx_out"][:, 0]
        out[idx] = res.results[c]["upd_out"]
    return out.reshape(B, T, D)


if __name__ == "__main__":
    import reference
    inp = {k: np.asarray(v) for k, v in reference.setup_inputs().items()}
    got = kernel(**inp)
    want = np.asarray(reference.reference(**reference.setup_inputs()))
    err = np.abs(got - want).max() / np.abs(want).max()
    print("rel err:", err)
